# revision 46
# baseline (speedup 1.0000x reference)
"""AdaptiveGraphConv Trainium2 kernel: 8-core SPMD, data-parallel over B.

Reference computation (per (b,t) slice over V=25 nodes):
  th = theta(x)+b, ph = phi(x)+b   (1x1 convs to INTER=32)
  A  = softmax(th @ ph / sqrt(INTER))    (V x V attention)
  out = A @ g(x)                         (g: 1x1 conv to C_OUT=128)
  BatchNorm2d (training stats over (B,T,V)) + affine.

Mapping (each core: B/8=4 batches, POS=30000 positions, 240 groups of
125 positions = 5 t-slices):

  scores[v,w] = x^_v^T Q x^_w  where x^ = [x; 1; e_0..e_4; 1] (71 rows)
  and Q (71x71, host-precomputed) folds theta^T phi, both biases, AND
  the block-diagonal softmax mask (+169 on slice-indicator diagonal,
  -169 via the duplicate ones row) so cross-slice scores exp to ~1e-13.

  Per 500-col chunk (4 groups):
    R = Q x^                       (PE, N=500, bf16)
    S_j = R_j^T x^_j               (PE, 4x N=125, bf16) -> scoresT [w,v]
    P = exp(SCALE * S)             (ACT, psum->sbuf bf16)
    M1_j = [xT_j ones]^T P_j       (PE, 4x N=125, bf16): rows 0:64 = x@A
                                    unnormalized, row 64 = Z (softmax denom)
    Y  = gw^T M1[0:64]             (PE, N=500, bf16): [C_OUT, pos] unnorm
    rz = exp(-ln(Z))               (ACT tables on the [1,500] Z row; the
                                    exact DVE reciprocal costs 7.4 cyc/elem)
    ZB = ones (x) rz               (PE, K=1 N=500): 1/Z broadcast to PSUM
    zb = copy(ZB)                  (DVE, psum->sbuf bf16)
    stash = Y*zb, accum col-sum    (DVE scalar_tensor_tensor)
  Sum of squares per channel accumulates via DVE scalar_tensor_tensor
  over stash superchunks interleaved with phase 1.  BN stats all-reduce
  ([128,2] floats), then per-channel affine fused into the fp32 output
  stream.

  g_b is dropped: rows of A sum to 1, so +g_b[o] is a per-channel shift
  that training-mode BN's mean subtraction cancels exactly.

Software pipeline: iteration k issues R(k), S(k-1), M1(k-2), Y/ZB/
norm(k-3), so no engine waits on same-iteration upstream results.
"""

import sys

sys.path.insert(0, "/opt/trn_rl_repo")

from contextlib import ExitStack

import numpy as np
import ml_dtypes

from concourse import bacc, bass, mybir, tile
from concourse.bass_utils import run_bass_kernel_spmd

B, C_IN, T, V = 32, 64, 300, 25
C_OUT, INTER = 128, 32
EPS = 1e-5
NCORES = 8
BPC = B // NCORES            # batches per core
POS = BPC * T * V            # 30000 positions per core
GP = 125                     # positions per PE group (5 t-slices)
NG = POS // GP               # 240 groups per core
CHUNK = 500                  # 4 groups per chunk
NCH = POS // CHUNK           # 60 chunks
SUP = 2500                   # superchunk (DMA granularity)
NSUP = POS // SUP            # 12
NT = B * T * V               # 240000 (BN denominator)
SCALE = 1.0 / float(np.sqrt(INTER))
XR = 71                      # x rows: 64 chans + ones + 5 indicators + ones
MR = 65                      # M1 rows: 64 chans + Z row
MBIG = 169.0                 # mask magnitude (exact in bf16)

F32 = mybir.dt.float32
BF16 = mybir.dt.bfloat16
F32R = mybir.dt.float32r
AF = mybir.ActivationFunctionType
ALU = mybir.AluOpType

_CACHE = {}


def _build(single_core=False):
    nc = bacc.Bacc(
        "TRN2",
        target_bir_lowering=False,
        debug=False,
        num_devices=1 if single_core else NCORES,
    )
    xe_d = nc.dram_tensor("xe", [XR, POS], BF16, kind="ExternalInput")
    xt_d = nc.dram_tensor("xt", [GP, NG * MR], BF16, kind="ExternalInput")
    qt_d = nc.dram_tensor("qt", [XR, XR], BF16, kind="ExternalInput")
    gw_d = nc.dram_tensor("gw", [C_IN, C_OUT], BF16, kind="ExternalInput")
    gb_d = nc.dram_tensor("gamma_beta", [C_OUT, 2], F32, kind="ExternalInput")
    out_d = nc.dram_tensor("out", [C_OUT, POS], F32, kind="ExternalOutput")

    with tile.TileContext(nc) as tc, ExitStack() as ctx:
        const = ctx.enter_context(tc.tile_pool(name="const", bufs=1))
        stash_p = ctx.enter_context(tc.tile_pool(name="stash", bufs=1))
        xsup_p = ctx.enter_context(tc.tile_pool(name="xsup", bufs=3))
        work = ctx.enter_context(tc.tile_pool(name="work", bufs=3))
        outp = ctx.enter_context(tc.tile_pool(name="outp", bufs=2))
        ps_r_p = ctx.enter_context(tc.tile_pool(name="psR", bufs=2, space="PSUM"))
        ps_s_p = ctx.enter_context(tc.tile_pool(name="psS", bufs=2, space="PSUM"))
        ps_m_p = ctx.enter_context(tc.tile_pool(name="psM", bufs=2, space="PSUM"))
        ps_y_p = ctx.enter_context(tc.tile_pool(name="psY", bufs=1, space="PSUM"))
        ps_b_p = ctx.enter_context(tc.tile_pool(name="psB", bufs=1, space="PSUM"))
        dram = ctx.enter_context(tc.tile_pool(name="dram", bufs=1, space="DRAM"))

        qt = const.tile([XR, XR], BF16)
        nc.sync.dma_start(qt[:], qt_d[:])
        gw = const.tile([C_IN, C_OUT], BF16)
        nc.sync.dma_start(gw[:], gw_d[:])
        gb = const.tile([C_OUT, 2], F32)
        nc.sync.dma_start(gb[:], gb_d[:])

        # K=1 stationary for broadcasting the 1/Z row (partition 0)
        ones0 = const.tile([1, C_OUT], BF16)
        nc.gpsimd.memset(ones0[:], 1.0)

        acc = const.tile([C_OUT, NCH], F32)
        sqacc = const.tile([C_OUT, NSUP], F32)
        sq_scratch = const.tile([C_OUT, SUP], BF16)

        stash = [stash_p.tile([C_OUT, SUP], BF16, name=f"st{s}") for s in range(NSUP)]

        # superchunk input tiles, prefetched 5 iterations ahead
        xe_t, xt_t = {}, {}

        def dma_sup(s):
            xe = xsup_p.tile([XR, SUP], BF16, tag="xe", name=f"xe{s}")
            nc.sync.dma_start(xe[:], xe_d[:, s * SUP : (s + 1) * SUP])
            xt = xsup_p.tile([GP, 20 * MR], BF16, tag="xt", name=f"xt{s}")
            nc.sync.dma_start(xt[:], xt_d[:, s * 20 * MR : (s + 1) * 20 * MR])
            xe_t[s], xt_t[s] = xe, xt

        dma_sup(0)

        r_sbs, pexps, m1s, rzs = {}, {}, {}, {}

        for k in range(NCH + 3):
            if k % 5 == 0 and k // 5 + 1 < NSUP:
                dma_sup(k // 5 + 1)
            # ---- stage D: Y(k-3), Z bcast+recip, normalize+stash+colsum
            # (emitted first: all inputs were produced in earlier iterations,
            # so the PE and DVE start each iteration without cross-engine
            # waits)
            c = k - 3
            if 0 <= c < NCH:
                m1 = m1s.pop(c)
                rzrow = rzs.pop(c)
                ps_b = ps_b_p.tile([C_OUT, CHUNK], F32)
                nc.tensor.matmul(
                    ps_b[:], ones0[:], rzrow[:], start=True, stop=True
                )
                ps_y = ps_y_p.tile([C_OUT, CHUNK], F32)
                nc.tensor.matmul(
                    ps_y[:], gw[:], m1[0:C_IN, :], start=True, stop=True
                )
                zb = work.tile([C_OUT, CHUNK], BF16, tag="zb", name=f"zb{c}")
                nc.vector.tensor_copy(zb[:], ps_b[:])
                s_idx, soff = c // 5, (c % 5) * CHUNK
                nc.vector.scalar_tensor_tensor(
                    out=stash[s_idx][:, soff : soff + CHUNK],
                    in0=ps_y[:],
                    scalar=1.0,
                    in1=zb[:],
                    op0=ALU.mult,
                    op1=ALU.mult,
                    accum_out=acc[:, c : c + 1],
                )
            # ---- sum-of-squares (DVE), one superchunk per 5 iterations ----
            if k >= 8 and (k - 8) % 5 == 0:
                s = (k - 8) // 5
                nc.vector.scalar_tensor_tensor(
                    out=sq_scratch[:],
                    in0=stash[s][:],
                    scalar=1.0,
                    in1=stash[s][:],
                    op0=ALU.mult,
                    op1=ALU.mult,
                    accum_out=sqacc[:, s : s + 1],
                )
            # ---- stage A: R(k) = Q @ x^ ----
            if k < NCH:
                xe = xe_t[k // 5]
                off = (k % 5) * CHUNK
                ps_r = ps_r_p.tile([XR, CHUNK], F32)
                nc.tensor.matmul(
                    ps_r[:], qt[:], xe[:, off : off + CHUNK], start=True, stop=True
                )
                r_sb = work.tile([XR, CHUNK], BF16, tag="r", name=f"r{k}")
                nc.vector.tensor_copy(r_sb[:], ps_r[:])
                r_sbs[k] = r_sb
            # ---- stage B: scoresT(k-1) + exp ----
            c = k - 1
            if 0 <= c < NCH:
                xe = xe_t[c // 5]
                off = (c % 5) * CHUNK
                r_sb = r_sbs.pop(c)
                ps_s = ps_s_p.tile([GP, CHUNK], F32)
                for j in range(4):
                    sl = slice(j * GP, (j + 1) * GP)
                    nc.tensor.matmul(
                        ps_s[:, sl],
                        r_sb[:, sl],
                        xe[:, off + j * GP : off + (j + 1) * GP],
                        start=True, stop=True,
                    )
                pexp = work.tile([GP, CHUNK], BF16, tag="pexp", name=f"p{c}")
                nc.scalar.activation(pexp[:], ps_s[:], AF.Exp, scale=SCALE)
                pexps[c] = pexp
            # ---- stage C: M1(k-2) = [xT ones]^T @ P (row 64 = Z) ----
            c = k - 2
            if 0 <= c < NCH:
                xt = xt_t[c // 5]
                pexp = pexps.pop(c)
                ps_m = ps_m_p.tile([MR, CHUNK], F32)
                for j in range(4):
                    gg = (c % 5) * 4 + j
                    nc.tensor.matmul(
                        ps_m[:, j * GP : (j + 1) * GP],
                        xt[:, gg * MR : (gg + 1) * MR],
                        pexp[:, j * GP : (j + 1) * GP],
                        start=True, stop=True,
                    )
                m1 = work.tile([MR, CHUNK], BF16, tag="m1", name=f"m{c}")
                nc.vector.tensor_copy(m1[:], ps_m[:])
                m1s[c] = m1
                # 1/Z row via exp(-ln(Z)) on the ACT table engine
                lnz = work.tile([1, CHUNK], F32, tag="lnz", name=f"ln{c}")
                nc.scalar.activation(
                    lnz[:], ps_m[C_IN : C_IN + 1, :], AF.Ln
                )
                rzrow = work.tile([1, CHUNK], BF16, tag="rzr", name=f"rr{c}")
                nc.scalar.activation(rzrow[:], lnz[:], AF.Exp, scale=-1.0)
                rzs[c] = rzrow

        nc.vector.scalar_tensor_tensor(
            out=sq_scratch[:],
            in0=stash[NSUP - 1][:],
            scalar=1.0,
            in1=stash[NSUP - 1][:],
            op0=ALU.mult,
            op1=ALU.mult,
            accum_out=sqacc[:, NSUP - 1 : NSUP],
        )

        # ---- phase 2: BN stats all-reduce + per-channel affine coefs ----
        cc_sb = work.tile([C_OUT, 2], F32, tag="ccsb")
        nc.vector.tensor_reduce(
            cc_sb[:, 0:1], acc[:], mybir.AxisListType.X, ALU.add
        )
        nc.vector.tensor_reduce(
            cc_sb[:, 1:2], sqacc[:], mybir.AxisListType.X, ALU.add
        )
        cc_in = dram.tile([C_OUT, 2], F32)
        cc_out = dram.tile([C_OUT, 2], F32)
        nc.sync.dma_start(cc_in[:], cc_sb[:])
        if single_core:
            nc.sync.dma_start(cc_out[:], cc_in[:])
        else:
            nc.gpsimd.collective_compute(
                "AllReduce",
                ALU.add,
                replica_groups=[list(range(NCORES))],
                ins=[cc_in.opt()],
                outs=[cc_out.opt()],
            )
        gstats = work.tile([C_OUT, 2], F32, tag="gstats")
        nc.sync.dma_start(gstats[:], cc_out[:])
        mcol = work.tile([C_OUT, 1], F32, tag="mcol")
        nc.vector.tensor_scalar_mul(mcol[:], gstats[:, 0:1], 1.0 / NT)
        vcol = work.tile([C_OUT, 1], F32, tag="vcol")
        nc.vector.tensor_scalar_mul(vcol[:], gstats[:, 1:2], 1.0 / NT)
        m2col = work.tile([C_OUT, 1], F32, tag="m2col")
        nc.scalar.square(m2col[:], mcol[:])
        nc.vector.tensor_sub(vcol[:], vcol[:], m2col[:])  # var = E[y^2]-mean^2
        nc.vector.tensor_scalar_add(vcol[:], vcol[:], float(EPS))
        sdcol = work.tile([C_OUT, 1], F32, tag="sdcol")
        nc.scalar.activation(sdcol[:], vcol[:], AF.Sqrt)
        scol = work.tile([C_OUT, 1], F32, tag="scol")
        nc.vector.reciprocal(scol[:], sdcol[:])            # rstd
        nc.vector.tensor_mul(scol[:], scol[:], gb[:, 0:1])  # s = gamma*rstd
        ccol = work.tile([C_OUT, 1], F32, tag="ccol")
        nc.vector.tensor_mul(ccol[:], mcol[:], scol[:])
        nc.vector.tensor_sub(ccol[:], gb[:, 1:2], ccol[:])  # c = beta - mean*s

        # ---- phase 3: BN affine fused into fp32 output stream ----
        for s in range(NSUP):
            ob = outp.tile([C_OUT, SUP], F32, tag="ob", name=f"ob{s}")
            if s % 2 == 0:
                nc.scalar.activation(
                    ob[:], stash[s][:], AF.Identity, bias=ccol[:], scale=scol[:]
                )
            else:
                nc.vector.tensor_scalar(
                    ob[:], stash[s][:], scol[:], ccol[:], ALU.mult, ALU.add
                )
            nc.sync.dma_start(out_d[:, s * SUP : (s + 1) * SUP], ob[:])

    nc.compile()
    return nc


def _host_prep(theta_w, theta_b, phi_w, phi_b, g_w, bn_gamma, bn_beta):
    th_hat = np.concatenate(
        [np.asarray(theta_w), np.asarray(theta_b)[:, None]], axis=1
    ).astype(np.float64)  # [32, 65]
    ph_hat = np.concatenate(
        [np.asarray(phi_w), np.asarray(phi_b)[:, None]], axis=1
    ).astype(np.float64)
    q = np.zeros((XR, XR), dtype=np.float64)
    q[:65, :65] = th_hat.T @ ph_hat
    for s in range(5):
        q[65 + s, 65 + s] = MBIG
    q[70, 70] = -MBIG
    qt = q.T.astype(ml_dtypes.bfloat16)
    gwT = np.asarray(g_w).T.astype(ml_dtypes.bfloat16).copy()  # [64, 128]
    gbmat = np.stack(
        [np.asarray(bn_gamma), np.asarray(bn_beta)], axis=1
    ).astype(np.float32)  # [128, 2]
    return qt, gwT, gbmat


def _per_core_inputs(xc):
    # xc: [64, POS] float32 for this core (b-major positions)
    pos = np.arange(POS)
    ind = ((pos // V) % 5)[None, :] == np.arange(5)[:, None]  # [5, POS]
    xe = np.concatenate(
        [
            xc,
            np.ones((1, POS), np.float32),
            ind.astype(np.float32),
            np.ones((1, POS), np.float32),
        ],
        axis=0,
    ).astype(ml_dtypes.bfloat16)  # [71, POS]
    xt = np.concatenate(
        [
            xc.reshape(C_IN, NG, GP).transpose(2, 1, 0),  # [125, 240, 64]
            np.ones((GP, NG, 1), np.float32),
        ],
        axis=2,
    ).reshape(GP, NG * MR).astype(ml_dtypes.bfloat16)
    return xe, xt


def kernel(x, theta_w, theta_b, phi_w, phi_b, g_w, g_b, bn_gamma, bn_beta):
    x = np.asarray(x, dtype=np.float32)
    if "nc" not in _CACHE:
        _CACHE["nc"] = _build()
    nc = _CACHE["nc"]

    qt, gwT, gbmat = _host_prep(
        theta_w, theta_b, phi_w, phi_b, g_w, bn_gamma, bn_beta
    )

    in_maps = []
    for c in range(NCORES):
        xc = (
            x[c * BPC : (c + 1) * BPC]
            .transpose(1, 0, 2, 3)
            .reshape(C_IN, POS)
        )
        xe, xt = _per_core_inputs(xc)
        in_maps.append(
            {
                "xe": xe,
                "xt": xt,
                "qt": qt,
                "gw": gwT,
                "gamma_beta": gbmat,
            }
        )

    res = run_bass_kernel_spmd(nc, in_maps, core_ids=list(range(NCORES)))
    out = np.empty((B, C_OUT, T, V), dtype=np.float32)
    for c in range(NCORES):
        oc = res.results[c]["out"]  # (C_OUT, POS), b-major positions
        out[c * BPC : (c + 1) * BPC] = (
            oc.reshape(C_OUT, BPC, T, V).transpose(1, 0, 2, 3)
        )
    return out


# revision 48
# speedup vs baseline: 1.0577x; 1.0577x over previous
"""AdaptiveGraphConv Trainium2 kernel: 8-core SPMD, data-parallel over B.

Reference computation (per (b,t) slice over V=25 nodes):
  th = theta(x)+b, ph = phi(x)+b   (1x1 convs to INTER=32)
  A  = softmax(th @ ph / sqrt(INTER))    (V x V attention)
  out = A @ g(x)                         (g: 1x1 conv to C_OUT=128)
  BatchNorm2d (training stats over (B,T,V)) + affine.

Mapping (each core: B/8=4 batches, POS=30000 positions, 240 groups of
125 positions = 5 t-slices):

  scores[v,w] = x^_v^T Q x^_w  where x^ = [x; 1; e_0..e_4; 1] (71 rows)
  and Q (71x71, host-precomputed) folds theta^T phi, both biases, AND
  the block-diagonal softmax mask (+169 on slice-indicator diagonal,
  -169 via the duplicate ones row) so cross-slice scores exp to ~1e-13.

  Per 500-col chunk (4 groups):
    R = Q x^                       (PE, N=500, bf16)
    S_j = R_j^T x^_j               (PE, 4x N=125, bf16) -> scoresT [w,v]
    P = exp(SCALE * S)             (ACT, psum->sbuf bf16)
    M1_j = [xT_j ones]^T P_j       (PE, 4x N=125, bf16): rows 0:64 = x@A
                                    unnormalized, row 64 = Z (softmax denom)
    Y  = gw^T M1[0:64]             (PE, N=500, bf16): [C_OUT, pos] unnorm
    rz = exp(-ln(Z))               (ACT tables on the [1,500] Z row; the
                                    exact DVE reciprocal costs 7.4 cyc/elem)
    ZB = ones (x) rz               (PE, K=1 N=500): 1/Z broadcast to PSUM
    zb = copy(ZB)                  (DVE, psum->sbuf bf16)
    stash = Y*zb, accum col-sum    (DVE scalar_tensor_tensor)
  Sum of squares per channel accumulates via DVE scalar_tensor_tensor
  over stash superchunks interleaved with phase 1.  BN stats all-reduce
  ([128,2] floats), then per-channel affine fused into the fp32 output
  stream.

  g_b is dropped: rows of A sum to 1, so +g_b[o] is a per-channel shift
  that training-mode BN's mean subtraction cancels exactly.

Software pipeline: iteration k issues R(k), S(k-1), M1(k-2), Y/ZB/
norm(k-3), so no engine waits on same-iteration upstream results.
"""

import sys

sys.path.insert(0, "/opt/trn_rl_repo")

from contextlib import ExitStack

import numpy as np
import ml_dtypes

from concourse import bacc, bass, mybir, tile
from concourse.bass_utils import run_bass_kernel_spmd

B, C_IN, T, V = 32, 64, 300, 25
C_OUT, INTER = 128, 32
EPS = 1e-5
NCORES = 8
BPC = B // NCORES            # batches per core
POS = BPC * T * V            # 30000 positions per core
GP = 125                     # positions per PE group (5 t-slices)
NG = POS // GP               # 240 groups per core
CHUNK = 500                  # 4 groups per chunk
NCH = POS // CHUNK           # 60 chunks
SUP = 2500                   # superchunk (DMA granularity)
NSUP = POS // SUP            # 12
NT = B * T * V               # 240000 (BN denominator)
SCALE = 1.0 / float(np.sqrt(INTER))
XR = 71                      # x rows: 64 chans + ones + 5 indicators + ones
MR = 65                      # M1 rows: 64 chans + Z row
MBIG = 169.0                 # mask magnitude (exact in bf16)

F32 = mybir.dt.float32
BF16 = mybir.dt.bfloat16
F32R = mybir.dt.float32r
AF = mybir.ActivationFunctionType
ALU = mybir.AluOpType

_CACHE = {}


def _build(single_core=False):
    nc = bacc.Bacc(
        "TRN2",
        target_bir_lowering=False,
        debug=False,
        num_devices=1 if single_core else NCORES,
    )
    xe_d = nc.dram_tensor("xe", [XR, POS], BF16, kind="ExternalInput")
    xt_d = nc.dram_tensor("xt", [GP, NG * MR], BF16, kind="ExternalInput")
    qt_d = nc.dram_tensor("qt", [XR, XR], BF16, kind="ExternalInput")
    gw_d = nc.dram_tensor("gw", [C_IN, C_OUT], BF16, kind="ExternalInput")
    gb_d = nc.dram_tensor("gamma_beta", [C_OUT, 2], F32, kind="ExternalInput")
    out_d = nc.dram_tensor("out", [C_OUT, POS], F32, kind="ExternalOutput")

    with tile.TileContext(nc) as tc, ExitStack() as ctx:
        const = ctx.enter_context(tc.tile_pool(name="const", bufs=1))
        stash_p = ctx.enter_context(tc.tile_pool(name="stash", bufs=1))
        xsup_p = ctx.enter_context(tc.tile_pool(name="xsup", bufs=3))
        work = ctx.enter_context(tc.tile_pool(name="work", bufs=3))
        outp = ctx.enter_context(tc.tile_pool(name="outp", bufs=2))
        ps_r_p = ctx.enter_context(tc.tile_pool(name="psR", bufs=2, space="PSUM"))
        ps_s_p = ctx.enter_context(tc.tile_pool(name="psS", bufs=2, space="PSUM"))
        ps_m_p = ctx.enter_context(tc.tile_pool(name="psM", bufs=2, space="PSUM"))
        ps_y_p = ctx.enter_context(tc.tile_pool(name="psY", bufs=1, space="PSUM"))
        ps_b_p = ctx.enter_context(tc.tile_pool(name="psB", bufs=1, space="PSUM"))
        dram = ctx.enter_context(tc.tile_pool(name="dram", bufs=1, space="DRAM"))

        qt = const.tile([XR, XR], BF16)
        nc.sync.dma_start(qt[:], qt_d[:])
        gw = const.tile([C_IN, C_OUT], BF16)
        nc.sync.dma_start(gw[:], gw_d[:])
        gb = const.tile([C_OUT, 2], F32)
        nc.sync.dma_start(gb[:], gb_d[:])

        # K=1 stationary for broadcasting the 1/Z row (partition 0)
        ones0 = const.tile([1, C_OUT], BF16)
        nc.gpsimd.memset(ones0[:], 1.0)

        acc = const.tile([C_OUT, NCH], F32)
        sqacc = const.tile([C_OUT, NSUP], F32)
        sq_scratch = const.tile([C_OUT, SUP], BF16)

        stash = [stash_p.tile([C_OUT, SUP], BF16, name=f"st{s}") for s in range(NSUP)]

        # superchunk input tiles, prefetched 5 iterations ahead
        xe_t, xt_t = {}, {}

        def dma_sup(s):
            xe = xsup_p.tile([XR, SUP], BF16, tag="xe", name=f"xe{s}")
            nc.sync.dma_start(xe[:], xe_d[:, s * SUP : (s + 1) * SUP])
            xt = xsup_p.tile([GP, 20 * MR], BF16, tag="xt", name=f"xt{s}")
            nc.sync.dma_start(xt[:], xt_d[:, s * 20 * MR : (s + 1) * 20 * MR])
            xe_t[s], xt_t[s] = xe, xt

        dma_sup(0)

        r_sbs, pexps, m1s, rzs = {}, {}, {}, {}

        for k in range(NCH + 3):
            if k % 5 == 0 and k // 5 + 1 < NSUP:
                dma_sup(k // 5 + 1)
            # ---- stage D: Y(k-3), Z bcast+recip, normalize+stash+colsum
            # (emitted first: all inputs were produced in earlier iterations,
            # so the PE and DVE start each iteration without cross-engine
            # waits)
            c = k - 3
            if 0 <= c < NCH:
                m1 = m1s.pop(c)
                rzrow = rzs.pop(c)
                ps_b = ps_b_p.tile([C_OUT, CHUNK], F32)
                nc.tensor.matmul(
                    ps_b[:], ones0[:], rzrow[:], start=True, stop=True
                )
                ps_y = ps_y_p.tile([C_OUT, CHUNK], F32)
                nc.tensor.matmul(
                    ps_y[:], gw[:], m1[0:C_IN, :], start=True, stop=True
                )
                zb = work.tile([C_OUT, CHUNK], BF16, tag="zb", name=f"zb{c}")
                nc.vector.tensor_copy(zb[:], ps_b[:])
                s_idx, soff = c // 5, (c % 5) * CHUNK
                nc.vector.scalar_tensor_tensor(
                    out=stash[s_idx][:, soff : soff + CHUNK],
                    in0=ps_y[:],
                    scalar=1.0,
                    in1=zb[:],
                    op0=ALU.mult,
                    op1=ALU.mult,
                    accum_out=acc[:, c : c + 1],
                )
            # ---- sum-of-squares (DVE), one superchunk per 5 iterations ----
            if k >= 8 and (k - 8) % 5 == 0:
                s = (k - 8) // 5
                nc.vector.scalar_tensor_tensor(
                    out=sq_scratch[:],
                    in0=stash[s][:],
                    scalar=1.0,
                    in1=stash[s][:],
                    op0=ALU.mult,
                    op1=ALU.mult,
                    accum_out=sqacc[:, s : s + 1],
                )
            # ---- stage A: R(k) = Q @ x^ ----
            if k < NCH:
                xe = xe_t[k // 5]
                off = (k % 5) * CHUNK
                ps_r = ps_r_p.tile([XR, CHUNK], F32)
                nc.tensor.matmul(
                    ps_r[:], qt[:], xe[:, off : off + CHUNK], start=True, stop=True
                )
                r_sb = work.tile([XR, CHUNK], BF16, tag="r", name=f"r{k}")
                nc.scalar.activation(r_sb[:], ps_r[:], AF.Copy)
                r_sbs[k] = r_sb
            # ---- stage B: scoresT(k-1) + exp ----
            c = k - 1
            if 0 <= c < NCH:
                xe = xe_t[c // 5]
                off = (c % 5) * CHUNK
                r_sb = r_sbs.pop(c)
                ps_s = ps_s_p.tile([GP, CHUNK], F32)
                for j in range(4):
                    sl = slice(j * GP, (j + 1) * GP)
                    nc.tensor.matmul(
                        ps_s[:, sl],
                        r_sb[:, sl],
                        xe[:, off + j * GP : off + (j + 1) * GP],
                        start=True, stop=True,
                    )
                pexp = work.tile([GP, CHUNK], BF16, tag="pexp", name=f"p{c}")
                nc.scalar.activation(pexp[:], ps_s[:], AF.Exp, scale=SCALE)
                pexps[c] = pexp
            # ---- stage C: M1(k-2) = [xT ones]^T @ P (row 64 = Z) ----
            c = k - 2
            if 0 <= c < NCH:
                xt = xt_t[c // 5]
                pexp = pexps.pop(c)
                ps_m = ps_m_p.tile([MR, CHUNK], F32)
                for j in range(4):
                    gg = (c % 5) * 4 + j
                    nc.tensor.matmul(
                        ps_m[:, j * GP : (j + 1) * GP],
                        xt[:, gg * MR : (gg + 1) * MR],
                        pexp[:, j * GP : (j + 1) * GP],
                        start=True, stop=True,
                    )
                m1 = work.tile([MR, CHUNK], BF16, tag="m1", name=f"m{c}")
                nc.vector.tensor_copy(m1[:], ps_m[:])
                m1s[c] = m1
                # 1/Z row via exp(-ln(Z)) on the ACT table engine
                lnz = work.tile([1, CHUNK], F32, tag="lnz", name=f"ln{c}")
                nc.scalar.activation(
                    lnz[:], ps_m[C_IN : C_IN + 1, :], AF.Ln
                )
                rzrow = work.tile([1, CHUNK], BF16, tag="rzr", name=f"rr{c}")
                nc.scalar.activation(rzrow[:], lnz[:], AF.Exp, scale=-1.0)
                rzs[c] = rzrow

        nc.vector.scalar_tensor_tensor(
            out=sq_scratch[:],
            in0=stash[NSUP - 1][:],
            scalar=1.0,
            in1=stash[NSUP - 1][:],
            op0=ALU.mult,
            op1=ALU.mult,
            accum_out=sqacc[:, NSUP - 1 : NSUP],
        )

        # ---- phase 2: BN stats all-reduce + per-channel affine coefs ----
        cc_sb = work.tile([C_OUT, 2], F32, tag="ccsb")
        nc.vector.tensor_reduce(
            cc_sb[:, 0:1], acc[:], mybir.AxisListType.X, ALU.add
        )
        nc.vector.tensor_reduce(
            cc_sb[:, 1:2], sqacc[:], mybir.AxisListType.X, ALU.add
        )
        cc_in = dram.tile([C_OUT, 2], F32)
        cc_out = dram.tile([C_OUT, 2], F32)
        nc.sync.dma_start(cc_in[:], cc_sb[:])
        if single_core:
            nc.sync.dma_start(cc_out[:], cc_in[:])
        else:
            nc.gpsimd.collective_compute(
                "AllReduce",
                ALU.add,
                replica_groups=[list(range(NCORES))],
                ins=[cc_in.opt()],
                outs=[cc_out.opt()],
            )
        gstats = work.tile([C_OUT, 2], F32, tag="gstats")
        nc.sync.dma_start(gstats[:], cc_out[:])
        mcol = work.tile([C_OUT, 1], F32, tag="mcol")
        nc.vector.tensor_scalar_mul(mcol[:], gstats[:, 0:1], 1.0 / NT)
        vcol = work.tile([C_OUT, 1], F32, tag="vcol")
        nc.vector.tensor_scalar_mul(vcol[:], gstats[:, 1:2], 1.0 / NT)
        m2col = work.tile([C_OUT, 1], F32, tag="m2col")
        nc.scalar.square(m2col[:], mcol[:])
        nc.vector.tensor_sub(vcol[:], vcol[:], m2col[:])  # var = E[y^2]-mean^2
        nc.vector.tensor_scalar_add(vcol[:], vcol[:], float(EPS))
        sdcol = work.tile([C_OUT, 1], F32, tag="sdcol")
        nc.scalar.activation(sdcol[:], vcol[:], AF.Sqrt)
        scol = work.tile([C_OUT, 1], F32, tag="scol")
        nc.vector.reciprocal(scol[:], sdcol[:])            # rstd
        nc.vector.tensor_mul(scol[:], scol[:], gb[:, 0:1])  # s = gamma*rstd
        ccol = work.tile([C_OUT, 1], F32, tag="ccol")
        nc.vector.tensor_mul(ccol[:], mcol[:], scol[:])
        nc.vector.tensor_sub(ccol[:], gb[:, 1:2], ccol[:])  # c = beta - mean*s

        # ---- phase 3: BN affine fused into fp32 output stream ----
        for s in range(NSUP):
            ob = outp.tile([C_OUT, SUP], F32, tag="ob", name=f"ob{s}")
            if s % 2 == 0:
                nc.scalar.activation(
                    ob[:], stash[s][:], AF.Identity, bias=ccol[:], scale=scol[:]
                )
                nc.sync.dma_start(out_d[:, s * SUP : (s + 1) * SUP], ob[:])
            else:
                nc.vector.tensor_scalar(
                    ob[:], stash[s][:], scol[:], ccol[:], ALU.mult, ALU.add
                )
                nc.scalar.dma_start(out_d[:, s * SUP : (s + 1) * SUP], ob[:])

    nc.compile()
    return nc


def _host_prep(theta_w, theta_b, phi_w, phi_b, g_w, bn_gamma, bn_beta):
    th_hat = np.concatenate(
        [np.asarray(theta_w), np.asarray(theta_b)[:, None]], axis=1
    ).astype(np.float64)  # [32, 65]
    ph_hat = np.concatenate(
        [np.asarray(phi_w), np.asarray(phi_b)[:, None]], axis=1
    ).astype(np.float64)
    q = np.zeros((XR, XR), dtype=np.float64)
    q[:65, :65] = th_hat.T @ ph_hat
    for s in range(5):
        q[65 + s, 65 + s] = MBIG
    q[70, 70] = -MBIG
    qt = q.T.astype(ml_dtypes.bfloat16)
    gwT = np.asarray(g_w).T.astype(ml_dtypes.bfloat16).copy()  # [64, 128]
    gbmat = np.stack(
        [np.asarray(bn_gamma), np.asarray(bn_beta)], axis=1
    ).astype(np.float32)  # [128, 2]
    return qt, gwT, gbmat


def _per_core_inputs(xc):
    # xc: [64, POS] float32 for this core (b-major positions)
    pos = np.arange(POS)
    ind = ((pos // V) % 5)[None, :] == np.arange(5)[:, None]  # [5, POS]
    xe = np.concatenate(
        [
            xc,
            np.ones((1, POS), np.float32),
            ind.astype(np.float32),
            np.ones((1, POS), np.float32),
        ],
        axis=0,
    ).astype(ml_dtypes.bfloat16)  # [71, POS]
    xt = np.concatenate(
        [
            xc.reshape(C_IN, NG, GP).transpose(2, 1, 0),  # [125, 240, 64]
            np.ones((GP, NG, 1), np.float32),
        ],
        axis=2,
    ).reshape(GP, NG * MR).astype(ml_dtypes.bfloat16)
    return xe, xt


def kernel(x, theta_w, theta_b, phi_w, phi_b, g_w, g_b, bn_gamma, bn_beta):
    x = np.asarray(x, dtype=np.float32)
    if "nc" not in _CACHE:
        _CACHE["nc"] = _build()
    nc = _CACHE["nc"]

    qt, gwT, gbmat = _host_prep(
        theta_w, theta_b, phi_w, phi_b, g_w, bn_gamma, bn_beta
    )

    in_maps = []
    for c in range(NCORES):
        xc = (
            x[c * BPC : (c + 1) * BPC]
            .transpose(1, 0, 2, 3)
            .reshape(C_IN, POS)
        )
        xe, xt = _per_core_inputs(xc)
        in_maps.append(
            {
                "xe": xe,
                "xt": xt,
                "qt": qt,
                "gw": gwT,
                "gamma_beta": gbmat,
            }
        )

    res = run_bass_kernel_spmd(nc, in_maps, core_ids=list(range(NCORES)))
    out = np.empty((B, C_OUT, T, V), dtype=np.float32)
    for c in range(NCORES):
        oc = res.results[c]["out"]  # (C_OUT, POS), b-major positions
        out[c * BPC : (c + 1) * BPC] = (
            oc.reshape(C_OUT, BPC, T, V).transpose(1, 0, 2, 3)
        )
    return out


# revision 49
# speedup vs baseline: 1.1020x; 1.0419x over previous
"""AdaptiveGraphConv Trainium2 kernel: 8-core SPMD, data-parallel over B.

Reference computation (per (b,t) slice over V=25 nodes):
  th = theta(x)+b, ph = phi(x)+b   (1x1 convs to INTER=32)
  A  = softmax(th @ ph / sqrt(INTER))    (V x V attention)
  out = A @ g(x)                         (g: 1x1 conv to C_OUT=128)
  BatchNorm2d (training stats over (B,T,V)) + affine.

Mapping (each core: B/8=4 batches, POS=30000 positions, 240 groups of
125 positions = 5 t-slices):

  scores[v,w] = x^_v^T Q x^_w  where x^ = [x; 1; e_0..e_4; 1] (71 rows)
  and Q (71x71, host-precomputed) folds theta^T phi, both biases, AND
  the block-diagonal softmax mask (+169 on slice-indicator diagonal,
  -169 via the duplicate ones row) so cross-slice scores exp to ~1e-13.

  Per 500-col chunk (4 groups):
    R = Q x^                       (PE, N=500, bf16)
    S_j = R_j^T x^_j               (PE, 4x N=125, bf16) -> scoresT [w,v]
    P = exp(SCALE * S)             (ACT, psum->sbuf bf16)
    M1_j = [xT_j ones]^T P_j       (PE, 4x N=125, bf16): rows 0:64 = x@A
                                    unnormalized, row 64 = Z (softmax denom)
    Y  = gw^T M1[0:64]             (PE, N=500, bf16): [C_OUT, pos] unnorm
    rz = exp(-ln(Z))               (ACT tables on the [1,500] Z row; the
                                    exact DVE reciprocal costs 7.4 cyc/elem)
    ZB = ones (x) rz               (PE, K=1 N=500): 1/Z broadcast to PSUM
    zb = copy(ZB)                  (DVE, psum->sbuf bf16)
    stash = Y*zb, accum col-sum    (DVE scalar_tensor_tensor)
  Sum of squares per channel accumulates via DVE scalar_tensor_tensor
  over stash superchunks interleaved with phase 1.  BN stats all-reduce
  ([128,2] floats), then per-channel affine fused into the fp32 output
  stream.

  g_b is dropped: rows of A sum to 1, so +g_b[o] is a per-channel shift
  that training-mode BN's mean subtraction cancels exactly.

Software pipeline: iteration k issues R(k), S(k-1), M1(k-2), Y/ZB/
norm(k-3), so no engine waits on same-iteration upstream results.
"""

import sys

sys.path.insert(0, "/opt/trn_rl_repo")

from contextlib import ExitStack

import numpy as np
import ml_dtypes

from concourse import bacc, bass, mybir, tile
from concourse.bass_utils import run_bass_kernel_spmd

B, C_IN, T, V = 32, 64, 300, 25
C_OUT, INTER = 128, 32
EPS = 1e-5
NCORES = 8
BPC = B // NCORES            # batches per core
POS = BPC * T * V            # 30000 positions per core
GP = 125                     # positions per PE group (5 t-slices)
NG = POS // GP               # 240 groups per core
CHUNK = 500                  # 4 groups per chunk
NCH = POS // CHUNK           # 60 chunks
SUP = 2500                   # superchunk (DMA granularity)
NSUP = POS // SUP            # 12
NT = B * T * V               # 240000 (BN denominator)
SCALE = 1.0 / float(np.sqrt(INTER))
XR = 71                      # x rows: 64 chans + ones + 5 indicators + ones
MR = 65                      # M1 rows: 64 chans + Z row
MBIG = 169.0                 # mask magnitude (exact in bf16)

F32 = mybir.dt.float32
BF16 = mybir.dt.bfloat16
F32R = mybir.dt.float32r
AF = mybir.ActivationFunctionType
ALU = mybir.AluOpType

_CACHE = {}


def _build(single_core=False):
    nc = bacc.Bacc(
        "TRN2",
        target_bir_lowering=False,
        debug=False,
        num_devices=1 if single_core else NCORES,
    )
    xe_d = nc.dram_tensor("xe", [XR, POS], BF16, kind="ExternalInput")
    xt_d = nc.dram_tensor("xt", [GP, NG * MR], BF16, kind="ExternalInput")
    qt_d = nc.dram_tensor("qt", [XR, XR], BF16, kind="ExternalInput")
    gw_d = nc.dram_tensor("gw", [C_IN, C_OUT], BF16, kind="ExternalInput")
    gb_d = nc.dram_tensor("gamma_beta", [C_OUT, 2], F32, kind="ExternalInput")
    out_d = nc.dram_tensor("out", [C_OUT, POS], F32, kind="ExternalOutput")

    with tile.TileContext(nc) as tc, ExitStack() as ctx:
        const = ctx.enter_context(tc.tile_pool(name="const", bufs=1))
        stash_p = ctx.enter_context(tc.tile_pool(name="stash", bufs=1))
        xsup_p = ctx.enter_context(tc.tile_pool(name="xsup", bufs=3))
        work = ctx.enter_context(tc.tile_pool(name="work", bufs=3))
        outp = ctx.enter_context(tc.tile_pool(name="outp", bufs=2))
        ps_r_p = ctx.enter_context(tc.tile_pool(name="psR", bufs=2, space="PSUM"))
        ps_s_p = ctx.enter_context(tc.tile_pool(name="psS", bufs=2, space="PSUM"))
        ps_m_p = ctx.enter_context(tc.tile_pool(name="psM", bufs=2, space="PSUM"))
        ps_y_p = ctx.enter_context(tc.tile_pool(name="psY", bufs=1, space="PSUM"))
        ps_b_p = ctx.enter_context(tc.tile_pool(name="psB", bufs=1, space="PSUM"))
        dram = ctx.enter_context(tc.tile_pool(name="dram", bufs=1, space="DRAM"))

        qt = const.tile([XR, XR], BF16)
        nc.sync.dma_start(qt[:], qt_d[:])
        gw = const.tile([C_IN, C_OUT], BF16)
        nc.sync.dma_start(gw[:], gw_d[:])
        gb = const.tile([C_OUT, 2], F32)
        nc.sync.dma_start(gb[:], gb_d[:])

        # K=1 stationary for broadcasting the 1/Z row (partition 0)
        ones0 = const.tile([1, C_OUT], BF16)
        nc.gpsimd.memset(ones0[:], 1.0)

        acc = const.tile([C_OUT, NCH], F32)
        sqacc = const.tile([C_OUT, NSUP], F32)
        sq_scratch = const.tile([C_OUT, SUP], BF16)

        stash = [stash_p.tile([C_OUT, SUP], BF16, name=f"st{s}") for s in range(NSUP)]

        # superchunk input tiles, prefetched 5 iterations ahead
        xe_t, xt_t = {}, {}

        def dma_sup(s):
            xe = xsup_p.tile([XR, SUP], BF16, tag="xe", name=f"xe{s}")
            nc.sync.dma_start(xe[:], xe_d[:, s * SUP : (s + 1) * SUP])
            xt = xsup_p.tile([GP, 20 * MR], BF16, tag="xt", name=f"xt{s}")
            nc.sync.dma_start(xt[:], xt_d[:, s * 20 * MR : (s + 1) * 20 * MR])
            xe_t[s], xt_t[s] = xe, xt

        dma_sup(0)

        r_sbs, pexps, m1s, rzs = {}, {}, {}, {}

        for k in range(NCH + 3):
            if k % 5 == 0 and k // 5 + 1 < NSUP:
                dma_sup(k // 5 + 1)
            # ---- stage D: Y(k-3), Z bcast+recip, normalize+stash+colsum
            # (emitted first: all inputs were produced in earlier iterations,
            # so the PE and DVE start each iteration without cross-engine
            # waits)
            c = k - 3
            if 0 <= c < NCH:
                m1 = m1s.pop(c)
                rzrow = rzs.pop(c)
                ps_b = ps_b_p.tile([C_OUT, CHUNK], F32)
                nc.tensor.matmul(
                    ps_b[:], ones0[:], rzrow[:], start=True, stop=True
                )
                ps_y = ps_y_p.tile([C_OUT, CHUNK], F32)
                nc.tensor.matmul(
                    ps_y[:], gw[:], m1[0:C_IN, :], start=True, stop=True
                )
                zb = work.tile([C_OUT, CHUNK], BF16, tag="zb", name=f"zb{c}")
                nc.vector.tensor_copy(zb[:], ps_b[:])
                s_idx, soff = c // 5, (c % 5) * CHUNK
                nc.vector.scalar_tensor_tensor(
                    out=stash[s_idx][:, soff : soff + CHUNK],
                    in0=ps_y[:],
                    scalar=1.0,
                    in1=zb[:],
                    op0=ALU.mult,
                    op1=ALU.mult,
                    accum_out=acc[:, c : c + 1],
                )
            # ---- sum-of-squares (DVE), one superchunk per 5 iterations ----
            if k >= 8 and (k - 8) % 5 == 0:
                s = (k - 8) // 5
                nc.vector.scalar_tensor_tensor(
                    out=sq_scratch[:],
                    in0=stash[s][:],
                    scalar=1.0,
                    in1=stash[s][:],
                    op0=ALU.mult,
                    op1=ALU.mult,
                    accum_out=sqacc[:, s : s + 1],
                )
            # ---- stage A: R(k) = Q @ x^ ----
            if k < NCH:
                xe = xe_t[k // 5]
                off = (k % 5) * CHUNK
                ps_r = ps_r_p.tile([XR, CHUNK], F32)
                nc.tensor.matmul(
                    ps_r[:], qt[:], xe[:, off : off + CHUNK], start=True, stop=True
                )
                r_sb = work.tile([XR, CHUNK], BF16, tag="r", name=f"r{k}")
                nc.scalar.activation(r_sb[:], ps_r[:], AF.Copy)
                r_sbs[k] = r_sb
            # ---- stage B: scoresT(k-1) + exp ----
            c = k - 1
            if 0 <= c < NCH:
                xe = xe_t[c // 5]
                off = (c % 5) * CHUNK
                r_sb = r_sbs.pop(c)
                ps_s = ps_s_p.tile([GP, CHUNK], F32)
                for j in range(4):
                    sl = slice(j * GP, (j + 1) * GP)
                    nc.tensor.matmul(
                        ps_s[:, sl],
                        r_sb[:, sl],
                        xe[:, off + j * GP : off + (j + 1) * GP],
                        start=True, stop=True,
                    )
                pexp = work.tile([GP, CHUNK], BF16, tag="pexp", name=f"p{c}")
                nc.scalar.activation(pexp[:], ps_s[:], AF.Exp, scale=SCALE)
                pexps[c] = pexp
            # ---- stage C: M1(k-2) = [xT ones]^T @ P (row 64 = Z) ----
            c = k - 2
            if 0 <= c < NCH:
                xt = xt_t[c // 5]
                pexp = pexps.pop(c)
                ps_m = ps_m_p.tile([MR, CHUNK], F32)
                for j in range(4):
                    gg = (c % 5) * 4 + j
                    nc.tensor.matmul(
                        ps_m[:, j * GP : (j + 1) * GP],
                        xt[:, gg * MR : (gg + 1) * MR],
                        pexp[:, j * GP : (j + 1) * GP],
                        start=True, stop=True,
                    )
                m1 = work.tile([MR, CHUNK], BF16, tag="m1", name=f"m{c}")
                nc.vector.tensor_copy(m1[:], ps_m[:])
                m1s[c] = m1
                # 1/Z row via exp(-ln(Z)) on the ACT table engine
                lnz = work.tile([1, CHUNK], F32, tag="lnz", name=f"ln{c}")
                nc.scalar.activation(
                    lnz[:], ps_m[C_IN : C_IN + 1, :], AF.Ln
                )
                rzrow = work.tile([1, CHUNK], BF16, tag="rzr", name=f"rr{c}")
                nc.scalar.activation(rzrow[:], lnz[:], AF.Exp, scale=-1.0)
                rzs[c] = rzrow

        nc.vector.scalar_tensor_tensor(
            out=sq_scratch[:],
            in0=stash[NSUP - 1][:],
            scalar=1.0,
            in1=stash[NSUP - 1][:],
            op0=ALU.mult,
            op1=ALU.mult,
            accum_out=sqacc[:, NSUP - 1 : NSUP],
        )

        # ---- phase 2: BN stats all-reduce + per-channel affine coefs ----
        cc_sb = work.tile([C_OUT, 2], F32, tag="ccsb")
        nc.vector.tensor_reduce(
            cc_sb[:, 0:1], acc[:], mybir.AxisListType.X, ALU.add
        )
        nc.vector.tensor_reduce(
            cc_sb[:, 1:2], sqacc[:], mybir.AxisListType.X, ALU.add
        )
        cc_in = dram.tile([C_OUT, 2], F32)
        cc_out = dram.tile([C_OUT, 2], F32)
        nc.sync.dma_start(cc_in[:], cc_sb[:])
        if single_core:
            nc.sync.dma_start(cc_out[:], cc_in[:])
        else:
            nc.gpsimd.collective_compute(
                "AllReduce",
                ALU.add,
                replica_groups=[list(range(NCORES))],
                ins=[cc_in.opt()],
                outs=[cc_out.opt()],
            )
        gstats = work.tile([C_OUT, 2], F32, tag="gstats")
        nc.sync.dma_start(gstats[:], cc_out[:])
        mcol = work.tile([C_OUT, 1], F32, tag="mcol")
        nc.vector.tensor_scalar_mul(mcol[:], gstats[:, 0:1], 1.0 / NT)
        vcol = work.tile([C_OUT, 1], F32, tag="vcol")
        nc.vector.tensor_scalar_mul(vcol[:], gstats[:, 1:2], 1.0 / NT)
        m2col = work.tile([C_OUT, 1], F32, tag="m2col")
        nc.scalar.square(m2col[:], mcol[:])
        nc.vector.tensor_sub(vcol[:], vcol[:], m2col[:])  # var = E[y^2]-mean^2
        nc.vector.tensor_scalar_add(vcol[:], vcol[:], float(EPS))
        sdcol = work.tile([C_OUT, 1], F32, tag="sdcol")
        nc.scalar.activation(sdcol[:], vcol[:], AF.Sqrt)
        scol = work.tile([C_OUT, 1], F32, tag="scol")
        nc.vector.reciprocal(scol[:], sdcol[:])            # rstd
        nc.vector.tensor_mul(scol[:], scol[:], gb[:, 0:1])  # s = gamma*rstd
        ccol = work.tile([C_OUT, 1], F32, tag="ccol")
        nc.vector.tensor_mul(ccol[:], mcol[:], scol[:])
        nc.vector.tensor_sub(ccol[:], gb[:, 1:2], ccol[:])  # c = beta - mean*s

        # ---- phase 3: BN affine fused into fp32 output stream ----
        for s in range(NSUP):
            ob = outp.tile([C_OUT, SUP], F32, tag="ob", name=f"ob{s}")
            if s % 2 == 0:
                nc.scalar.activation(
                    ob[:], stash[s][:], AF.Identity, bias=ccol[:], scale=scol[:]
                )
            else:
                nc.vector.tensor_scalar(
                    ob[:], stash[s][:], scol[:], ccol[:], ALU.mult, ALU.add
                )
            nc.sync.dma_start(out_d[:, s * SUP : (s + 1) * SUP], ob[:])

    nc.compile()
    return nc


def _host_prep(theta_w, theta_b, phi_w, phi_b, g_w, bn_gamma, bn_beta):
    th_hat = np.concatenate(
        [np.asarray(theta_w), np.asarray(theta_b)[:, None]], axis=1
    ).astype(np.float64)  # [32, 65]
    ph_hat = np.concatenate(
        [np.asarray(phi_w), np.asarray(phi_b)[:, None]], axis=1
    ).astype(np.float64)
    q = np.zeros((XR, XR), dtype=np.float64)
    q[:65, :65] = th_hat.T @ ph_hat
    for s in range(5):
        q[65 + s, 65 + s] = MBIG
    q[70, 70] = -MBIG
    qt = q.T.astype(ml_dtypes.bfloat16)
    gwT = np.asarray(g_w).T.astype(ml_dtypes.bfloat16).copy()  # [64, 128]
    gbmat = np.stack(
        [np.asarray(bn_gamma), np.asarray(bn_beta)], axis=1
    ).astype(np.float32)  # [128, 2]
    return qt, gwT, gbmat


def _per_core_inputs(xc):
    # xc: [64, POS] float32 for this core (b-major positions)
    pos = np.arange(POS)
    ind = ((pos // V) % 5)[None, :] == np.arange(5)[:, None]  # [5, POS]
    xe = np.concatenate(
        [
            xc,
            np.ones((1, POS), np.float32),
            ind.astype(np.float32),
            np.ones((1, POS), np.float32),
        ],
        axis=0,
    ).astype(ml_dtypes.bfloat16)  # [71, POS]
    xt = np.concatenate(
        [
            xc.reshape(C_IN, NG, GP).transpose(2, 1, 0),  # [125, 240, 64]
            np.ones((GP, NG, 1), np.float32),
        ],
        axis=2,
    ).reshape(GP, NG * MR).astype(ml_dtypes.bfloat16)
    return xe, xt


def kernel(x, theta_w, theta_b, phi_w, phi_b, g_w, g_b, bn_gamma, bn_beta):
    x = np.asarray(x, dtype=np.float32)
    if "nc" not in _CACHE:
        _CACHE["nc"] = _build()
    nc = _CACHE["nc"]

    qt, gwT, gbmat = _host_prep(
        theta_w, theta_b, phi_w, phi_b, g_w, bn_gamma, bn_beta
    )

    in_maps = []
    for c in range(NCORES):
        xc = (
            x[c * BPC : (c + 1) * BPC]
            .transpose(1, 0, 2, 3)
            .reshape(C_IN, POS)
        )
        xe, xt = _per_core_inputs(xc)
        in_maps.append(
            {
                "xe": xe,
                "xt": xt,
                "qt": qt,
                "gw": gwT,
                "gamma_beta": gbmat,
            }
        )

    res = run_bass_kernel_spmd(nc, in_maps, core_ids=list(range(NCORES)))
    out = np.empty((B, C_OUT, T, V), dtype=np.float32)
    for c in range(NCORES):
        oc = res.results[c]["out"]  # (C_OUT, POS), b-major positions
        out[c * BPC : (c + 1) * BPC] = (
            oc.reshape(C_OUT, BPC, T, V).transpose(1, 0, 2, 3)
        )
    return out
